# revision 1
# baseline (speedup 1.0000x reference)
"""Sparse neighbor attention (N=50000, K=16, HIDDEN=256, 8 heads x 32) on
8 Trainium2 NeuronCores via Bass.

Sharding: nodes split evenly across the 8 cores (data-parallel, padded to
whole 128-node tiles); the keys/values table is replicated to every core
since neighbor_idx references arbitrary nodes.

Per-core program, per 128-node tile:
  - gather each node's 16 neighbor KV rows into its SBUF partition
    (16 indirect DMAs with one row-offset per partition)
  - scores[k,h] = sum_d q[h,d]*k[k,h,d]: DVE fp16 multiply + tree-reduce
  - softmax over k without max-subtraction (scores are O(1): q pre-scaled)
  - out[h,d] = sum_k w[k,h]*v[k,h,d]: ACT broadcast-expand of the weights,
    DVE multiply + tree-reduce, final level accumulated in fp32
"""
import os
import numpy as np

import concourse.bacc as bacc
import concourse.tile as tile
from concourse import bass, mybir
from concourse.bass_utils import run_bass_kernel_spmd

P = 128
K = 16
H = 8
D = 32
HID = 256          # H*D
ROW = 2 * HID      # interleaved K|V row, elements
N = 50000
NCORES = 8
PER = N // NCORES            # 6250 nodes per core
TAB_PHYS = 82768             # hybrid table size for int16 gather (see below)
NT = -(-PER // P)            # 49 tiles (padded to 6272)
GATHER_MODE = os.environ.get("ATTN_GATHER_MODE", "indirect16")
GATHER_SPLIT = int(os.environ.get("ATTN_GATHER_SPLIT", "4"))
DT_NP = np.float16
DT = mybir.dt.float16

LAST_EXEC_NS = None
LAST_RESULT = None
_CACHE = {}


def _view(ap, dims, offset=0):
    return bass.AP(ap.tensor, ap.offset + offset,
                   [ap.ap[0]] + [[s, c] for s, c in dims])


def _build_program(n_tiles, tab_rows, dt=DT, gather_mode=GATHER_MODE):
    f32 = mybir.dt.float32
    nc = bacc.Bacc("TRN2", target_bir_lowering=False, debug=False)
    # dma_gather mode: the gather ucode sign-extends the int16 index and adds
    # it to the base row (verified on HW). With the base at physical row 32768,
    # idx<32768 reads base+idx and idx>=32768 reads idx-32768, so the hybrid
    # table stores the logical rows at both places.
    kv_rows = TAB_PHYS if gather_mode == "dma_gather" else tab_rows
    kv_d = nc.dram_tensor("kv", [kv_rows, ROW], dt, kind="ExternalInput").ap()
    q_d = nc.dram_tensor("q", [n_tiles * P, HID], dt, kind="ExternalInput").ap()
    if gather_mode == "indirect16":
        idx_d = nc.dram_tensor("idx", [P, n_tiles * K], mybir.dt.int32,
                               kind="ExternalInput").ap()
    else:
        idx_d = nc.dram_tensor("idx", [P, n_tiles * P], mybir.dt.int16,
                               kind="ExternalInput").ap()
    out_d = nc.dram_tensor("out", [n_tiles * P, HID], f32,
                           kind="ExternalOutput").ap()

    with tile.TileContext(nc) as tc:
        with (
            tc.tile_pool(name="idxp", bufs=1) as idxp,
            tc.tile_pool(name="kvp", bufs=int(os.environ.get("ATTN_KV_BUFS", "3"))) as kvp,
            tc.tile_pool(name="qp", bufs=3) as qp,
            tc.tile_pool(name="scratch", bufs=2) as sp,
            tc.tile_pool(name="outp", bufs=3) as op_,
        ):
            if gather_mode == "indirect16":
                idx_all = idxp.tile([P, n_tiles * K], mybir.dt.int32)
            else:
                idx_all = idxp.tile([P, n_tiles * P], mybir.dt.int16)
            nc.sync.dma_start(out=idx_all[:], in_=idx_d[:])

            for t in range(n_tiles):
                kv = kvp.tile([P, K * ROW], dt, tag="kv")
                if gather_mode == "indirect16":
                    for k in range(K):
                        nc.gpsimd.indirect_dma_start(
                            out=kv[:, k * ROW:(k + 1) * ROW],
                            out_offset=None,
                            in_=kv_d,
                            in_offset=bass.IndirectOffsetOnAxis(
                                ap=idx_all[:, t * K + k:t * K + k + 1], axis=0),
                        )
                else:
                    # split into GATHER_SPLIT calls of <=1024 idxs each so the
                    # default single-packet mode (reliable completion sem) stays
                    # within the 64-descriptors-per-engine packet limit
                    ng = GATHER_SPLIT
                    rows_per = K * P // ng          # gathered rows per call
                    cols_per = rows_per // 16       # idx tile cols per call
                    chunks_per = rows_per // P      # dest ROW-chunks per call
                    for g in range(ng):
                        nc.gpsimd.dma_gather(
                            out_ap=_view(kv[:], [(ROW, chunks_per), (1, ROW)],
                                         offset=g * chunks_per * ROW),
                            in_ap=kv_d[32768:, :],
                            idxs_ap=idx_all[:, t * P + g * cols_per:
                                            t * P + (g + 1) * cols_per],
                            num_idxs=rows_per,
                            num_idxs_reg=rows_per,
                            elem_size=ROW,
                            single_packet=rows_per <= 1024,
                        )
                q = qp.tile([P, HID], dt, tag="q")
                nc.sync.dma_start(out=q[:], in_=q_d[t * P:(t + 1) * P, :])

                # scores products: tmp[k,h,d] = kv_K[k,h,d] * q[h,d]
                tmp = sp.tile([P, K * HID], dt, tag="tmp")
                nc.vector.tensor_tensor(out=tmp[:],
                                        in0=_view(kv[:], [(ROW, K), (1, HID)]),
                                        in1=_view(q[:], [(0, K), (1, HID)]),
                                        op=mybir.AluOpType.mult)
                # tree-reduce over d -> scores col = k*8+h
                cur, w = tmp, D
                while w > 2:
                    nxt = sp.tile([P, K * H * (w // 2)], dt, tag=f"red{w}")
                    nc.vector.tensor_tensor(
                        out=nxt[:],
                        in0=_view(cur[:], [(w, K * H), (1, w // 2)]),
                        in1=_view(cur[:], [(w, K * H), (1, w // 2)], offset=w // 2),
                        op=mybir.AluOpType.add)
                    cur, w = nxt, w // 2
                scores = sp.tile([P, K * H], f32, tag="scores")
                nc.vector.tensor_tensor(
                    out=scores[:],
                    in0=_view(cur[:], [(2, K * H), (1, 1)]),
                    in1=_view(cur[:], [(2, K * H), (1, 1)], offset=1),
                    op=mybir.AluOpType.add)

                # softmax over k
                e = sp.tile([P, K * H], dt, tag="e")
                nc.scalar.activation(out=e[:], in_=scores[:],
                                     func=mybir.ActivationFunctionType.Exp)
                den = sp.tile([P, H], f32, tag="den")
                nc.vector.tensor_reduce(
                    out=den[:], in_=_view(e[:], [(1, H), (H, K)]),
                    axis=mybir.AxisListType.X, op=mybir.AluOpType.add)
                r32 = sp.tile([P, H], f32, tag="r32")
                nc.vector.reciprocal(out=r32[:], in_=den[:])
                r16 = sp.tile([P, H], dt, tag="r16")
                nc.vector.tensor_copy(out=r16[:], in_=r32[:])
                en = sp.tile([P, K * H], dt, tag="en")
                nc.vector.tensor_tensor(out=en[:], in0=e[:],
                                        in1=_view(r16[:], [(0, K), (1, H)]),
                                        op=mybir.AluOpType.mult)

                # weighted V
                ex = sp.tile([P, K * HID], dt, tag="ex")
                nc.scalar.activation(
                    out=ex[:], in_=_view(en[:], [(H, K), (1, H), (0, D)]),
                    func=mybir.ActivationFunctionType.Copy)
                vw = sp.tile([P, K * HID], dt, tag="vw")
                nc.vector.tensor_tensor(out=vw[:],
                                        in0=_view(kv[:], [(ROW, K), (1, HID)],
                                                  offset=HID),
                                        in1=ex[:], op=mybir.AluOpType.mult)
                cur, w = vw, K
                while w > 2:
                    nxt = sp.tile([P, (w // 2) * HID], dt, tag=f"vred{w}")
                    nc.vector.tensor_tensor(
                        out=nxt[:],
                        in0=_view(cur[:], [(1, (w // 2) * HID)]),
                        in1=_view(cur[:], [(1, (w // 2) * HID)],
                                  offset=(w // 2) * HID),
                        op=mybir.AluOpType.add)
                    cur, w = nxt, w // 2
                of = op_.tile([P, HID], mybir.dt.float32, tag="of")
                nc.vector.tensor_tensor(
                    out=of[:], in0=_view(cur[:], [(1, HID)]),
                    in1=_view(cur[:], [(1, HID)], offset=HID),
                    op=mybir.AluOpType.add)

                nc.sync.dma_start(out=out_d[t * P:(t + 1) * P, :], in_=of[:])

    nc.compile()
    return nc


def _host_prep(keys, queries, values, neighbor_idx, gather_mode=GATHER_MODE):
    kv = np.concatenate([keys, values], axis=1).astype(DT_NP)   # [N, 512]
    if gather_mode == "dma_gather":
        rot = np.zeros((TAB_PHYS, ROW), DT_NP)
        rot[32768:32768 + N] = kv
        rot[0:N - 32768] = kv[32768:N]
        kv = rot
    qs = (queries.astype(np.float32) * (D ** -0.5)).astype(DT_NP)
    pad = NT * P
    in_maps = []
    for c in range(NCORES):
        q_c = np.zeros((pad, HID), DT_NP)
        q_c[:PER] = qs[c * PER:(c + 1) * PER]
        idx_c = np.zeros((pad, K), np.int64)
        idx_c[:PER] = neighbor_idx[c * PER:(c + 1) * PER]
        if gather_mode == "indirect16":
            idx_l = np.ascontiguousarray(
                idx_c.reshape(NT, P, K).transpose(1, 0, 2)
                .reshape(P, NT * K)).astype(np.int32)
        else:
            ng = GATHER_SPLIT
            rows_per = K * P // ng
            cols_per = rows_per // 16
            idx_l = np.zeros((P, NT * P), np.int16)
            for t in range(NT):
                flat = idx_c[t * P:(t + 1) * P].astype(np.uint16).view(np.int16)
                flat = flat.T.reshape(K * P)    # slot j = k*128+p
                for g in range(ng):
                    wrp = flat[g * rows_per:(g + 1) * rows_per]\
                        .reshape(cols_per, 16).T
                    for rep in range(8):
                        idx_l[rep * 16:(rep + 1) * 16,
                              t * P + g * cols_per:
                              t * P + (g + 1) * cols_per] = wrp
        in_maps.append({"kv": kv, "q": q_c, "idx": idx_l})
    return in_maps


def kernel(keys, queries, values, neighbor_idx):
    global LAST_EXEC_NS
    key = ("prog", NT, N, GATHER_MODE)
    if key not in _CACHE:
        _CACHE[key] = _build_program(NT, N)
    nc = _CACHE[key]
    in_maps = _host_prep(keys, queries, values, neighbor_idx)
    trace = bool(int(os.environ.get("ATTN_TRACE", "0")))
    res = run_bass_kernel_spmd(nc, in_maps, list(range(NCORES)), trace=trace)
    global LAST_RESULT
    LAST_RESULT = res
    LAST_EXEC_NS = res.exec_time_ns
    out = np.concatenate([np.asarray(res.results[c]["out"])[:PER]
                          for c in range(NCORES)], axis=0)
    return out.astype(np.float32)

